# revision 24
# baseline (speedup 1.0000x reference)
"""Causal self-attention (B=2, T=2048, C=768, H=12) on 8 TRN2 NeuronCores.

Sharding: core c = (b = c // 4, head-group hg = c % 4 of 3 heads).
Each core: QKV projection for its 3 heads (column-parallel), causal
attention, and a row-parallel slice of the output projection. The host
pre-transposes/casts operands to bf16 and sums the 4 partial outputs
per batch (row-parallel all-reduce done host-side) + bias.
"""

import os
import sys

import numpy as np
import ml_dtypes


def _ensure_paths():
    for p in ("/opt/trn_rl_repo", "/opt/pypackages"):
        if os.path.isdir(p) and p not in sys.path:
            sys.path.append(p)


_ensure_paths()

import concourse.bass as bass  # noqa: E402
import concourse.mybir as mybir  # noqa: E402
import concourse.tile as tile  # noqa: E402
from concourse import bacc  # noqa: E402
from concourse.bass_utils import run_bass_kernel_spmd  # noqa: E402
from concourse.masks import make_identity  # noqa: E402

BF16 = ml_dtypes.bfloat16

B, T, C, H, D = 2, 2048, 768, 12, 64
G = 3                # heads per core
FQK = 512            # q(192) | pad(64) | k(192) | pad(64) -> q/k same partition offsets
FV = G * D           # 192
NT = T // 128        # 16 token tiles
KS = C // 128        # 6 contraction subtiles

_cache: dict[bool, object] = {}
_last_in_maps = None


def _build(causal: bool):
    dt = mybir.dt
    nc = bacc.Bacc("TRN2", num_devices=8)

    xT_d = nc.dram_tensor("xT", [C, T], dt.bfloat16, kind="ExternalInput")
    wqkT_d = nc.dram_tensor("wqkT", [C, FQK], dt.bfloat16, kind="ExternalInput")
    wvT_d = nc.dram_tensor("wvT", [C, FV], dt.bfloat16, kind="ExternalInput")
    bqk_d = nc.dram_tensor("bqk", [128, 4], dt.float32, kind="ExternalInput")
    bv_d = nc.dram_tensor("bv", [128, FV], dt.float32, kind="ExternalInput")
    wpT_d = nc.dram_tensor("wpT", [256, C], dt.bfloat16, kind="ExternalInput")
    maskT_d = nc.dram_tensor("maskT", [128, 128], dt.bfloat16, kind="ExternalInput")
    out_d = nc.dram_tensor("out", [T, C], dt.float32, kind="ExternalOutput")

    Exp = mybir.ActivationFunctionType.Exp
    Log = mybir.ActivationFunctionType.Ln

    with tile.TileContext(nc) as tc:
        with tc.tile_pool(name="persist", bufs=1) as pp:
            xT_sb = pp.tile([128, KS, T], dt.bfloat16)
            wqkT_sb = pp.tile([128, KS, FQK], dt.bfloat16)
            wvT_sb = pp.tile([128, KS, FV], dt.bfloat16)
            wpT_sb = pp.tile([128, 2, C], dt.bfloat16)
            bqk_sb = pp.tile([128, 4], dt.float32)
            bv_sb = pp.tile([128, FV], dt.float32)
            maskT_sb = pp.tile([128, 128], dt.bfloat16)
            ones_sb = pp.tile([128, 64], dt.bfloat16)
            ident = pp.tile([128, 128], dt.bfloat16)
            y_sb = pp.tile([128, NT, FV], dt.bfloat16)
            qkT_sb = pp.tile([128, 4, T], dt.bfloat16)
            v_sb = pp.tile([128, NT, G, D + 1], dt.bfloat16)
            yT_sb = pp.tile([128, 2, T], dt.bfloat16)

            for s in range(KS):
                nc.sync.dma_start(
                    xT_sb[:, s, :],
                    xT_d.ap()[s * 128 : (s + 1) * 128, :],
                )
            for s in range(KS):
                nc.sync.dma_start(
                    wqkT_sb[:, s, :],
                    wqkT_d.ap()[s * 128 : (s + 1) * 128, :],
                )
            nc.sync.dma_start(
                wvT_sb[:], wvT_d.ap().rearrange("(s p) f -> p s f", p=128)
            )
            nc.sync.dma_start(
                wpT_sb[:], wpT_d.ap().rearrange("(s p) o -> p s o", p=128)
            )
            nc.sync.dma_start(bqk_sb[:], bqk_d.ap())
            nc.sync.dma_start(bv_sb[:], bv_d.ap())
            nc.sync.dma_start(maskT_sb[:], maskT_d.ap())
            nc.gpsimd.memset(ones_sb[:], 1.0)
            make_identity(nc, ident[:])

            # ---- Phase 1a: q/k projection -> qkT_sb [f, t] (bf16, +bias) ----
            with tc.tile_pool(name="ps_qk", bufs=3, space="PSUM") as qkps:
                for fi in (0, 2):
                    for tch in range(4):
                        ps = qkps.tile([128, 512], dt.float32)
                        for s in range(KS):
                            nc.tensor.matmul(
                                ps[:],
                                wqkT_sb[:, s, fi * 128 : (fi + 1) * 128],
                                xT_sb[:, s, tch * 512 : (tch + 1) * 512],
                                start=(s == 0),
                                stop=(s == KS - 1),
                            )
                        nc.vector.tensor_scalar_add(
                            qkT_sb[:, fi, tch * 512 : (tch + 1) * 512],
                            ps[:],
                            bqk_sb[:, fi : fi + 1],
                        )

            # ---- Phase 1b: v projection -> v_aug [t, g, d|1] (bf16, +bias) ----
            nc.gpsimd.memset(v_sb[:, :, :, D : D + 1], 1.0)
            with tc.tile_pool(name="ps_v", bufs=2, space="PSUM") as vps:
                for ti in range(NT):
                    ps = vps.tile([128, FV], dt.float32)
                    for s in range(KS):
                        nc.tensor.matmul(
                            ps[:],
                            xT_sb[:, s, ti * 128 : (ti + 1) * 128],
                            wvT_sb[:, s, :],
                            start=(s == 0),
                            stop=(s == KS - 1),
                        )
                    for h in range(G):
                        nc.vector.tensor_tensor(
                            v_sb[:, ti, h, 0:D],
                            ps[:, h * D : (h + 1) * D],
                            bv_sb[:, h * D : (h + 1) * D],
                            mybir.AluOpType.add,
                        )

            # ---- Phase 2: attention per head, q-tile outer (fine-grained) ----
            # scores transposed [j, q]; 4 j-blocks share one 1-bank psum and
            # one exp; PV: lhsT = pT block [j, q], rhs = v_aug -> y[q,:]+denom.
            # The y->yT transposes ride the h1/h2 loops and the out-projection
            # + DMA for t-tile qi run right after h2's qi epilogue, so the
            # whole tail overlaps attention and keeps the PE streams dense.
            nc.gpsimd.memset(yT_sb[64:128, 1, :], 0.0)
            with (
                tc.tile_pool(name="ps_s", bufs=2, space="PSUM") as sps,
                tc.tile_pool(name="ps_y", bufs=2, space="PSUM") as yps,
                tc.tile_pool(name="ps_tr", bufs=1, space="PSUM") as trp,
                tc.tile_pool(name="ps_o", bufs=1, space="PSUM") as ops_,
                tc.tile_pool(name="ps_qk2", bufs=2, space="PSUM") as qk2ps,
                tc.tile_pool(name="pt", bufs=4) as ptp,
                tc.tile_pool(name="eps", bufs=4) as ep,
                tc.tile_pool(name="ob", bufs=3) as obp,
            ):
                deferred_qk = [(fi, tch) for fi in (1, 3) for tch in range(4)]
                for h in range(G):
                    qf = 64 * h
                    kf = 256 + 64 * h
                    qti, qoff = qf // 128, qf % 128
                    kti, koff = kf // 128, kf % 128
                    for qi in range(NT):
                        if h == 0 and qi < len(deferred_qk):
                            dfi, dtch = deferred_qk[qi]
                            dps = qk2ps.tile([128, 512], dt.float32)
                            for s in range(KS):
                                nc.tensor.matmul(
                                    dps[:],
                                    wqkT_sb[:, s, dfi * 128 : (dfi + 1) * 128],
                                    xT_sb[:, s, dtch * 512 : (dtch + 1) * 512],
                                    start=(s == 0),
                                    stop=(s == KS - 1),
                                )
                            nc.vector.tensor_scalar_add(
                                qkT_sb[:, dfi, dtch * 512 : (dtch + 1) * 512],
                                dps[:],
                                bqk_sb[:, dfi : dfi + 1],
                            )
                        jmax = qi if causal else NT - 1
                        ypt = yps.tile([128, D + 1], dt.float32)
                        for g0 in range(0, jmax + 1, 4):
                            g1 = min(g0 + 4, jmax + 1)
                            ncols = (g1 - g0) * 128
                            sp = sps.tile([128, 512], dt.float32)
                            for j in range(g0, g1):
                                jj = j - g0
                                nc.tensor.matmul(
                                    sp[:, jj * 128 : (jj + 1) * 128],
                                    qkT_sb[koff : koff + 64, kti, j * 128 : (j + 1) * 128],
                                    qkT_sb[qoff : qoff + 64, qti, qi * 128 : (qi + 1) * 128],
                                    start=(jj == 0),
                                    stop=(jj == 3 or j == g1 - 1),
                                )
                            pt = ptp.tile([128, 512], dt.bfloat16)
                            nc.scalar.activation(
                                pt[:, 0:ncols], sp[:, 0:ncols], Exp, scale=0.125
                            )
                            if causal and g0 <= qi < g1:
                                jj = qi - g0
                                nc.vector.tensor_mul(
                                    pt[:, jj * 128 : (jj + 1) * 128],
                                    pt[:, jj * 128 : (jj + 1) * 128],
                                    maskT_sb[:],
                                )
                            for j in range(g0, g1):
                                jj = j - g0
                                nc.tensor.matmul(
                                    ypt[:],
                                    pt[:, jj * 128 : (jj + 1) * 128],
                                    v_sb[:, j, h, :],
                                    start=(j == 0),
                                    stop=(j == jmax),
                                )
                        rc = ep.tile([128, 1], dt.float32)
                        nc.vector.reciprocal(rc[:], ypt[:, D : D + 1])
                        nc.vector.tensor_scalar_mul(
                            y_sb[:, qi, h * D : (h + 1) * D],
                            ypt[:, 0:D],
                            rc[:, 0:1],
                        )
                        if h == 1:
                            # h0+h1 channel block of t-tile qi is complete
                            p1 = trp.tile([128, 128], dt.bfloat16, tag="tr")
                            nc.tensor.transpose(p1[:], y_sb[:, qi, 0:128], ident[:])
                            nc.vector.tensor_copy(
                                yT_sb[:, 0, qi * 128 : (qi + 1) * 128], p1[:]
                            )
                        elif h == 2:
                            p2 = trp.tile([128, 128], dt.bfloat16, tag="tr")
                            nc.tensor.transpose(
                                p2[0:64, :], y_sb[:, qi, 128:192], ident[:]
                            )
                            nc.vector.tensor_copy(
                                yT_sb[0:64, 1, qi * 128 : (qi + 1) * 128],
                                p2[0:64, :],
                            )
                            ob = obp.tile([128, C], dt.float32)
                            po1 = ops_.tile([128, 384], dt.float32, tag="po1")
                            for s in range(2):
                                nc.tensor.matmul(
                                    po1[:],
                                    yT_sb[:, s, qi * 128 : (qi + 1) * 128],
                                    wpT_sb[:, s, 0:384],
                                    start=(s == 0),
                                    stop=(s == 1),
                                )
                            nc.vector.tensor_copy(ob[:, 0:384], po1[:])
                            po2 = ops_.tile([128, 384], dt.float32, tag="po1", name="po2")
                            for s in range(2):
                                nc.tensor.matmul(
                                    po2[:],
                                    yT_sb[:, s, qi * 128 : (qi + 1) * 128],
                                    wpT_sb[:, s, 384:768],
                                    start=(s == 0),
                                    stop=(s == 1),
                                )
                            nc.vector.tensor_copy(ob[:, 384:768], po2[:])
                            nc.sync.dma_start(
                                out_d.ap()[qi * 128 : (qi + 1) * 128, :], ob[:]
                            )

    nc.compile()
    return nc


def _prep_in_maps(x, Wqkv, bqkv, Wproj):
    in_maps = []
    for c in range(8):
        b, hg = c // 4, c % 4
        r0 = 192 * hg
        xT = np.ascontiguousarray(x[b].T).astype(BF16)
        wqk = np.zeros((512, 768), dtype=np.float32)
        wqk[0:192] = Wqkv[r0 : r0 + 192]
        wqk[256:448] = Wqkv[768 + r0 : 768 + r0 + 192]
        wqkT = np.ascontiguousarray(wqk.T).astype(BF16)
        wvT = np.ascontiguousarray(Wqkv[1536 + r0 : 1536 + r0 + 192].T).astype(BF16)
        bqk_vec = np.zeros(512, dtype=np.float32)
        bqk_vec[0:192] = bqkv[r0 : r0 + 192]
        bqk_vec[256:448] = bqkv[768 + r0 : 768 + r0 + 192]
        bqk = np.ascontiguousarray(bqk_vec.reshape(4, 128).T.astype(np.float32))
        bv = np.tile(
            bqkv[1536 + r0 : 1536 + r0 + 192].astype(np.float32)[None, :], (128, 1)
        )
        wp = np.zeros((256, 768), dtype=BF16)
        wp[0:192] = Wproj[:, r0 : r0 + 192].T.astype(BF16)
        maskT = np.triu(np.ones((128, 128), dtype=np.float32)).astype(BF16)
        in_maps.append(
            {
                "xT": xT,
                "wqkT": np.ascontiguousarray(wqkT),
                "wvT": wvT,
                "bqk": np.ascontiguousarray(bqk),
                "bv": bv,
                "wpT": wp,
                "maskT": maskT,
            }
        )
    return in_maps


def kernel(x, Wqkv, bqkv, Wproj, bproj, is_causal):
    global _last_in_maps
    x = np.asarray(x, dtype=np.float32)
    Wqkv = np.asarray(Wqkv, dtype=np.float32)
    bqkv = np.asarray(bqkv, dtype=np.float32)
    Wproj = np.asarray(Wproj, dtype=np.float32)
    bproj = np.asarray(bproj, dtype=np.float32)
    causal = bool(int(np.asarray(is_causal)))

    if causal not in _cache:
        _cache[causal] = _build(causal)
    nc = _cache[causal]

    in_maps = _prep_in_maps(x, Wqkv, bqkv, Wproj)
    _last_in_maps = in_maps
    res = run_bass_kernel_spmd(nc, in_maps, core_ids=list(range(8)))

    out = np.empty((B, T, C), dtype=np.float32)
    for b in range(B):
        acc = res.results[4 * b]["out"].copy()
        for k in range(1, 4):
            acc += res.results[4 * b + k]["out"]
        out[b] = acc + bproj[None, :]
    return out
